# revision 20
# baseline (speedup 1.0000x reference)
"""Expert-parallel batched SwiGLU FFN for Trainium2 (8 NeuronCores, Bass/Tile).

Problem: out[e] = (silu(x[e] @ w1[e].T + b1[e]) * (x[e] @ w3[e].T + b3[e])) @ w2[e].T + b2[e]
with E=8, T=512, D_IN=7168, D_INT=2048, fp32 reference.

Sharding: expert-parallel - core e owns expert e end-to-end, no communication.

v2 strategy (from the v1 trace): v1 ran fp32r everywhere, putting HBM traffic
(209 MB/core = 584 us at 358 GB/s) right on top of the PE roofline (2688
512-col matmuls = 580 us), so every DMA hiccup was a PE stall (39 us of gaps +
7 HAM re-throttles). v2 stages x/w1/w3/w2 as bf16 (halves DMA to ~114 MB =
~320 us, same PE rate, rel err ~5e-4 -> ~4e-3, well under the 2e-2 gate) so the
PE is the sole critical engine.

v3 (from the v2 trace, 658us run: steady state is a clean 216 ns/MM; losses
were 12.6us stall at ft~5 + 20us HAM K=4/8 re-throttle it triggered (the 3.67MB
b2 broadcast injected into both weight queues), ~9us of startup gaps, ~12us
tail, plus ~27us of uncontrollable GPIO power-throttle):
  - b2 broadcast chunked onto the scalar queue behind phase-1 silu ACTs
    (time-gated by PE progress, lands in the HBM-slack half of phase 1)
  - b1/b3 on the idle gpsimd queue
  - HAM warmup: 24 zero matmuls during the preamble/first-DMA dead window
  - startup x chunks 0-1 split per-subtile for DMA-granularity chasing
  - phase 2: w2 halves split sync/scalar, outputs on the gpsimd queue

v4 (from the v3 trace, 636us run: tail is near-floor at ~7us; the loss is
~25us of startup gaps + 5 HAM K=4/8 oscillations in the first 90us — with 4
interleaved startup groups the PE demands ~304 GB/s while the two queues
deliver ~260 GB/s during the x load):
  - NPRE=3: 6 interleaved startup groups drop demand to ~252 GB/s (matched)
  - PSUM rebalance: phase-1 ring 6 banks, phase-2 per-512 groups (2 banks)

Layouts are host-swizzled so every DMA is per-partition contiguous:
  xs  [128][o][t]        o = d//128 (56), partition = d%128
  w1s/w3s [128][ft][o][fl]  ft = f//128 (16)
  w2s [128][db][fo][dw]  db = d//1024 (7), fo = f//128, dw = d%1024

Per-core schedule:
  phase 1: x fully SBUF-resident (56 KiB/part bf16). For each (ft, w in
    {w1,w3}): one 56-deep PSUM accumulation group (stationary = w tile
    [128d,128f], moving = x [128d,512t]); silu+bias straight off PSUM on
    ScalarE, then gt[ft] = (h3+b3)*s1 on DVE, written bf16.
    Startup: the ft0+ft1 group quadruple is interleaved chunk-by-chunk with
    the x DMA stream (8 chunks of 7 d-subtiles) so the PE starts ~3.5 us in
    instead of waiting ~20 us for all of x.
  phase 2: out[t,d] = sum_f gt[f,t]*w2[f,d]. gt tiles stationary, w2 moving,
    16-deep PSUM groups over fo into [128,1024] (2-bank) tiles per (db,ts);
    DVE adds b2 (host-broadcast) and output DMAs stream out per (db,ts).
    w2 blocks ride queue-S only, output DMAs queue-A only, so a ring-gated
    weight DMA can never head-of-line-block an output transfer.
"""

import numpy as np
import ml_dtypes

import concourse.bacc as bacc
import concourse.mybir as mybir
import concourse.tile as tile
from concourse.bass_utils import run_bass_kernel_spmd

# Problem shape (hardcoded per contest contract).
E = 8
T = 512
D = 7168
F = 2048
P = 128

DO = D // P  # 56 d-subtiles
FO = F // P  # 16 f-subtiles
TT = T // P  # 4 t-subtiles

XC = 8  # x chunks (phase-1 startup granularity)
XCW = DO // XC  # 7 d-subtiles per x chunk
NPRE = 3  # ft groups interleaved with the x stream at startup
HW_ = 28  # phase-1 steady-state weight half-block width (d-subtiles)
DBW = 1024  # phase-2 block width (d)
NDB = D // DBW  # 7 blocks
HFO = FO // 2  # 8: phase-2 w2 half block (fo)

PS1B = 2 * NPRE  # phase-1 PSUM ring (one bank per startup group; 8-bank cap)

F32 = mybir.dt.float32
BF16 = mybir.dt.bfloat16
BF = ml_dtypes.bfloat16

_NC = None


def _emit(nc, xs, w1s, w3s, w2s, b1, b3, b2r, out):
    add = mybir.AluOpType.add
    mult = mybir.AluOpType.mult
    silu = mybir.ActivationFunctionType.Silu

    xs_r = xs.ap().rearrange("p (o t) -> p o t", o=DO)  # [P, DO, T]
    w1s_r = w1s.ap().rearrange("p (ft x) -> p ft x", ft=FO)  # [P, FO, DO*P]
    w3s_r = w3s.ap().rearrange("p (ft x) -> p ft x", ft=FO)
    w2s_r = w2s.ap().rearrange("p (db x) -> p db x", db=NDB)  # [P, NDB, FO*DBW]
    b1_r = b1.ap().rearrange("(o p) -> p o", p=P)  # [P, FO]
    b3_r = b3.ap().rearrange("(o p) -> p o", p=P)
    out_r = out.ap().rearrange("(o p) d -> p o d", p=P)  # [P, TT, D]

    with tile.TileContext(nc) as tc:
        with (
            tc.tile_pool(name="persist", bufs=1) as persist,
            tc.tile_pool(name="wsp", bufs=1) as wsp,
            tc.tile_pool(name="evp", bufs=1) as evp,
            tc.tile_pool(name="psp", bufs=1, space="PSUM") as psp,
        ):
            gt = persist.tile([P, FO, T], BF16, tag="gt")
            b2s = persist.tile([P, D], F32, tag="b2s")
            b1s = persist.tile([P, FO], F32, tag="b1s")
            b3s = persist.tile([P, FO], F32, tag="b3s")
            scr = persist.tile([P, 2 * P], BF16, tag="scr")
            xch = [
                persist.tile([P, XCW, T], BF16, tag=f"xc{c}", name=f"xc{c}")
                for c in range(XC)
            ]

            # ---- HAM warmup: the framework preamble pins the PE idle until
            # ~7us and the first real matmul can't start before its DMAs land
            # (~11.5us). Fill that window with zero matmuls on a DVE-memset
            # scratch tile so the HAM clock gate is already at 8/8 when the
            # real stream begins (cold MMs run at 1.2 GHz otherwise), and the
            # 3.4us busy-window has been paid off the critical path.
            nc.vector.memset(scr[:], 0.0)
            ps_warm = psp.tile([P, T], F32, tag="ps1", bufs=PS1B, name="ps_warm")
            NWARM = 24
            for i in range(NWARM):
                nc.tensor.matmul(
                    ps_warm[:, :P],
                    scr[:, :P],
                    scr[:, P:],
                    start=(i == 0),
                    stop=(i == NWARM - 1),
                )

            # ---- phase-1 startup DMA stream: x chunks + ft0/ft1 w quarters,
            # interleaved so the PE can chase the stream chunk by chunk. The
            # wq ring is deep (12 = 3 chunks of lookahead) so ring-gated
            # quarters can't convoy-block later x chunks in queue order.
            # Fixed queue roles during startup: all x on sync (7.34 MB, fully
            # resident by ~28us), all 32 w-quarters on scalar in exact
            # PE-consumption order — each queue's delivery order matches the
            # PE's need order, so neither can starve the other's stream.
            wq = {}  # (wi, ft, c) -> quarter tile [P, XCW*P]
            # (wi, ft) startup group order; 2*NPRE groups interleaved with the
            # x stream so PE demand (~252 GB/s at 6 groups) matches the
            # ~260 GB/s the two queues actually deliver during the x load —
            # at 4 groups demand was 304 GB/s and the PE starved + HAM
            # oscillated K=8/8 -> 4/8 five times in the first 90us.
            pre = [(wi, ft) for ft in range(NPRE) for wi in range(2)]
            for c in range(XC):
                if c < 2:
                    # per-subtile slivers: each matmul j is gated only on its
                    # own 131 KB sliver, so the PE chases the x stream at DMA
                    # granularity instead of stalling on whole 0.92 MB chunks
                    for j in range(XCW):
                        nc.sync.dma_start(
                            xch[c][:, j, :], xs_r[:, c * XCW + j, :]
                        )
                else:
                    nc.sync.dma_start(
                        xch[c][:], xs_r[:, c * XCW : (c + 1) * XCW, :]
                    )
                for wi, ft in pre:
                    wsrc = w1s_r if wi == 0 else w3s_r
                    q = wsp.tile([P, XCW * P], BF16, tag="wq", bufs=12, name="wq")
                    nc.scalar.dma_start(
                        q[:], wsrc[:, ft, c * XCW * P : (c + 1) * XCW * P]
                    )
                    wq[(wi, ft, c)] = q
            # biases are tiny (8 KB); ride the otherwise-idle gpsimd queue so
            # they never contend with the x/weight streams. Consumers (the
            # part-A evictions) are emitted below, so Tile sequences them.
            nc.gpsimd.dma_start(b1s[:], b1_r)
            nc.gpsimd.dma_start(b3s[:], b3_r)

            # ---- phase-1 startup matmuls: 4 groups interleaved chunk-wise.
            ps_pre = {}
            for wi, ft in pre:
                ps_pre[(wi, ft)] = psp.tile(
                    [P, T], F32, tag="ps1", bufs=PS1B, name="ps1"
                )
            for c in range(XC):
                for wi, ft in pre:
                    q = wq[(wi, ft, c)]
                    for j in range(XCW):
                        nc.tensor.matmul(
                            ps_pre[(wi, ft)][:],
                            q[:, j * P : (j + 1) * P],
                            xch[c][:, j, :],
                            start=(c == 0 and j == 0),
                            stop=(c == XC - 1 and j == XCW - 1),
                        )

            s1_cur = {}  # ft -> s1 tile

            def evict_p1(wi, ft, ps):
                if wi == 0:
                    s1 = evp.tile([P, T], F32, tag="s1", bufs=2, name="s1")
                    nc.scalar.activation(
                        s1[:], ps[:], silu, bias=b1s[:, ft : ft + 1]
                    )
                    s1_cur[ft] = s1
                else:
                    nc.vector.scalar_tensor_tensor(
                        out=gt[:, ft, :],
                        in0=ps[:],
                        scalar=b3s[:, ft : ft + 1],
                        in1=s1_cur[ft][:],
                        op0=add,
                        op1=mult,
                    )

            for wi, ft in pre:
                evict_p1(wi, ft, ps_pre[(wi, ft)])

            # ---- phase-1 steady state: ft 2..15, full 56-deep groups.
            # ft2/ft3 halves ride directly behind the part-A stream so they
            # land before the PE drains part A; biases after them; b2s later.
            halves = {}
            for ft in range(NPRE, FO):
                for wi in range(2):
                    wsrc = w1s_r if wi == 0 else w3s_r
                    hA = wsp.tile([P, HW_ * P], BF16, tag="wh", bufs=4, name="whA")
                    hB = wsp.tile([P, HW_ * P], BF16, tag="wh", bufs=4, name="whB")
                    nc.sync.dma_start(hA[:], wsrc[:, ft, : HW_ * P])
                    nc.scalar.dma_start(hB[:], wsrc[:, ft, HW_ * P :])
                    halves[(wi, ft)] = (hA, hB)

            for ft in range(NPRE, FO):
                for wi in range(2):
                    hA, hB = halves[(wi, ft)]
                    ps = psp.tile([P, T], F32, tag="ps1", bufs=PS1B, name="ps1")
                    for o in range(DO):
                        if o < HW_:
                            lhsT = hA[:, o * P : (o + 1) * P]
                        else:
                            lhsT = hB[:, (o - HW_) * P : (o - HW_ + 1) * P]
                        nc.tensor.matmul(
                            ps[:],
                            lhsT,
                            xch[o // XCW][:, o % XCW, :],
                            start=(o == 0),
                            stop=(o == DO - 1),
                        )
                    evict_p1(wi, ft, ps)
                # b2 broadcast (3.67 MB) in two 256 KB chunks per late ft
                # group on the scalar queue: the trigger instructions sit
                # behind the silu ACT in the scalar engine's FIFO, so the
                # transfers are time-gated by PE progress and land in the
                # second half of phase 1 where HBM has ~200 GB/s of slack
                # (injecting it into the weight queues at ft==5 caused a
                # 12.6us PE stall + a 20us HAM re-throttle in v2).
                if 8 <= ft <= 14:
                    for cc in (2 * (ft - 8), 2 * (ft - 8) + 1):
                        nc.scalar.dma_start(
                            b2s[:, cc * 512 : (cc + 1) * 512],
                            b2r.ap()[:, cc * 512 : (cc + 1) * 512],
                        )

            # ---------------- phase 2 ----------------
            # w2 halves split across sync+scalar (76 GB/s each instead of
            # 152 GB/s on sync alone); output DMAs ride the gpsimd queue
            # (their triggers carry semaphore waits on the DVE adds, and
            # gpsimd has nothing else in phase 2) so a ring-gated weight DMA
            # can never head-of-line-block them.
            w2h = {}
            for db in range(NDB):
                hA = wsp.tile([P, HFO * DBW], BF16, tag="w2h", bufs=3, name="w2hA")
                hB = wsp.tile([P, HFO * DBW], BF16, tag="w2h", bufs=3, name="w2hB")
                nc.sync.dma_start(hA[:], w2s_r[:, db, : HFO * DBW])
                nc.scalar.dma_start(hB[:], w2s_r[:, db, HFO * DBW :])
                w2h[db] = (hA, hB)

            # per-512 PSUM groups (2 banks total) leave room for the 6-bank
            # phase-1 ring; evictions + output DMAs go per 512 columns. All
            # output triggers ride gpsimd — sync/scalar carry the ring-gated
            # w2 triggers whose semaphore waits would head-of-line-block any
            # output trigger emitted behind them. Exception: the very last
            # transfer goes to sync (its w2 ring waits have long resolved by
            # then) so the final two 256 KB transfers overlap.
            for db in range(NDB):
                hA, hB = w2h[db]
                for ts in range(TT):
                    for half in range(2):
                        off = half * 512
                        ps2 = psp.tile([P, 512], F32, tag="ps2", bufs=2, name="ps2")
                        lhsT = gt[:, :, ts * P : (ts + 1) * P]
                        for fo in range(FO):
                            wh = hA if fo < HFO else hB
                            base = (fo % HFO) * DBW + off
                            nc.tensor.matmul(
                                ps2[:],
                                lhsT[:, fo, :],
                                wh[:, base : base + 512],
                                start=(fo == 0),
                                stop=(fo == FO - 1),
                            )
                        ost = evp.tile([P, 512], F32, tag="ost", bufs=2, name="ost")
                        dlo = db * DBW + off
                        nc.vector.tensor_add(ost[:], ps2[:], b2s[:, dlo : dlo + 512])
                        last = db == NDB - 1 and ts == TT - 1 and half == 1
                        q = nc.sync if last else nc.gpsimd
                        q.dma_start(out_r[:, ts, dlo : dlo + 512], ost[:])


def build():
    global _NC
    if _NC is not None:
        return _NC
    nc = bacc.Bacc("TRN2", target_bir_lowering=False, debug=False, num_devices=E)
    xs = nc.dram_tensor("xs", [P, DO * T], BF16, kind="ExternalInput")
    w1s = nc.dram_tensor("w1s", [P, FO * DO * P], BF16, kind="ExternalInput")
    w3s = nc.dram_tensor("w3s", [P, FO * DO * P], BF16, kind="ExternalInput")
    w2s = nc.dram_tensor("w2s", [P, NDB * FO * DBW], BF16, kind="ExternalInput")
    b1 = nc.dram_tensor("b1", [F], F32, kind="ExternalInput")
    b3 = nc.dram_tensor("b3", [F], F32, kind="ExternalInput")
    b2r = nc.dram_tensor("b2r", [P, D], F32, kind="ExternalInput")
    out = nc.dram_tensor("out", [T, D], F32, kind="ExternalOutput")
    _emit(nc, xs, w1s, w3s, w2s, b1, b3, b2r, out)
    nc.compile()
    _NC = nc
    return nc


def make_in_maps(x, w1, b1, w3, b3, w2, b2):
    x = np.asarray(x, dtype=np.float32)
    w1 = np.asarray(w1, dtype=np.float32)
    b1 = np.asarray(b1, dtype=np.float32)
    w3 = np.asarray(w3, dtype=np.float32)
    b3 = np.asarray(b3, dtype=np.float32)
    w2 = np.asarray(w2, dtype=np.float32)
    b2 = np.asarray(b2, dtype=np.float32)
    in_maps = []
    for e in range(E):
        # xs[p][o][t] = x[e][t, o*128+p]
        xs = x[e].reshape(T, DO, P).transpose(2, 1, 0).reshape(P, -1).astype(BF)
        # w1s[p][ft][o][fl] = w1[e][ft*128+fl, o*128+p]
        w1s = (
            w1[e].reshape(FO, P, DO, P).transpose(3, 0, 2, 1).reshape(P, -1).astype(BF)
        )
        w3s = (
            w3[e].reshape(FO, P, DO, P).transpose(3, 0, 2, 1).reshape(P, -1).astype(BF)
        )
        # w2s[p][db][fo][dw] = w2[e][db*1024+dw, fo*128+p]
        w2s = (
            w2[e]
            .reshape(NDB, DBW, FO, P)
            .transpose(3, 0, 2, 1)
            .reshape(P, -1)
            .astype(BF)
        )
        in_maps.append(
            {
                "xs": xs,
                "w1s": w1s,
                "w3s": w3s,
                "w2s": w2s,
                "b1": b1[e],
                "b3": b3[e],
                "b2r": np.ascontiguousarray(np.broadcast_to(b2[e], (P, D))),
            }
        )
    return in_maps


def run(x, w1, b1, w3, b3, w2, b2, **spmd_kwargs):
    nc = build()
    in_maps = make_in_maps(x, w1, b1, w3, b3, w2, b2)
    res = run_bass_kernel_spmd(nc, in_maps, core_ids=list(range(E)), **spmd_kwargs)
    out = np.stack([res.results[e]["out"] for e in range(E)], axis=0)
    return out, res


def kernel(x, w1, b1, w3, b3, w2, b2):
    out, _ = run(x, w1, b1, w3, b3, w2, b2)
    return out



# revision 23
# speedup vs baseline: 1.0462x; 1.0462x over previous
"""Expert-parallel batched SwiGLU FFN for Trainium2 (8 NeuronCores, Bass/Tile).

Problem: out[e] = (silu(x[e] @ w1[e].T + b1[e]) * (x[e] @ w3[e].T + b3[e])) @ w2[e].T + b2[e]
with E=8, T=512, D_IN=7168, D_INT=2048, fp32 reference.

Sharding: expert-parallel - core e owns expert e end-to-end, no communication.

v2 strategy (from the v1 trace): v1 ran fp32r everywhere, putting HBM traffic
(209 MB/core = 584 us at 358 GB/s) right on top of the PE roofline (2688
512-col matmuls = 580 us), so every DMA hiccup was a PE stall (39 us of gaps +
7 HAM re-throttles). v2 stages x/w1/w3/w2 as bf16 (halves DMA to ~114 MB =
~320 us, same PE rate, rel err ~5e-4 -> ~4e-3, well under the 2e-2 gate) so the
PE is the sole critical engine.

v3 (from the v2 trace, 658us run: steady state is a clean 216 ns/MM; losses
were 12.6us stall at ft~5 + 20us HAM K=4/8 re-throttle it triggered (the 3.67MB
b2 broadcast injected into both weight queues), ~9us of startup gaps, ~12us
tail, plus ~27us of uncontrollable GPIO power-throttle):
  - b2 broadcast chunked onto the scalar queue behind phase-1 silu ACTs
    (time-gated by PE progress, lands in the HBM-slack half of phase 1)
  - b1/b3 on the idle gpsimd queue
  - HAM warmup: 24 zero matmuls during the preamble/first-DMA dead window
  - startup x chunks 0-1 split per-subtile for DMA-granularity chasing
  - phase 2: w2 halves split sync/scalar, outputs on the gpsimd queue

v4 (from the v3 trace, 636us run: tail is near-floor at ~7us; the loss is
~25us of startup gaps + 5 HAM K=4/8 oscillations in the first 90us — with 4
interleaved startup groups the PE demands ~304 GB/s while the two queues
deliver ~260 GB/s during the x load):
  - NPRE=3: 6 interleaved startup groups drop demand to ~252 GB/s (matched)
  - PSUM rebalance: phase-1 ring 6 banks, phase-2 per-512 groups (2 banks)

Layouts are host-swizzled so every DMA is per-partition contiguous:
  xs  [128][o][t]        o = d//128 (56), partition = d%128
  w1s/w3s [128][ft][o][fl]  ft = f//128 (16)
  w2s [128][db][fo][dw]  db = d//1024 (7), fo = f//128, dw = d%1024

Per-core schedule:
  phase 1: x fully SBUF-resident (56 KiB/part bf16). For each (ft, w in
    {w1,w3}): one 56-deep PSUM accumulation group (stationary = w tile
    [128d,128f], moving = x [128d,512t]); silu+bias straight off PSUM on
    ScalarE, then gt[ft] = (h3+b3)*s1 on DVE, written bf16.
    Startup: the ft0+ft1 group quadruple is interleaved chunk-by-chunk with
    the x DMA stream (8 chunks of 7 d-subtiles) so the PE starts ~3.5 us in
    instead of waiting ~20 us for all of x.
  phase 2: out[t,d] = sum_f gt[f,t]*w2[f,d]. gt tiles stationary, w2 moving,
    16-deep PSUM groups over fo into [128,1024] (2-bank) tiles per (db,ts);
    DVE adds b2 (host-broadcast) and output DMAs stream out per (db,ts).
    w2 blocks ride queue-S only, output DMAs queue-A only, so a ring-gated
    weight DMA can never head-of-line-block an output transfer.
"""

import numpy as np
import ml_dtypes

import concourse.bacc as bacc
import concourse.mybir as mybir
import concourse.tile as tile
from concourse.bass_utils import run_bass_kernel_spmd

# Problem shape (hardcoded per contest contract).
E = 8
T = 512
D = 7168
F = 2048
P = 128

DO = D // P  # 56 d-subtiles
FO = F // P  # 16 f-subtiles
TT = T // P  # 4 t-subtiles

XC = 8  # x chunks (phase-1 startup granularity)
XCW = DO // XC  # 7 d-subtiles per x chunk
NPRE = 3  # ft groups interleaved with the x stream at startup
HW_ = 28  # phase-1 steady-state weight half-block width (d-subtiles)
DBW = 1024  # phase-2 block width (d)
NDB = D // DBW  # 7 blocks
HFO = FO // 2  # 8: phase-2 w2 half block (fo)

PS1B = 2 * NPRE  # phase-1 PSUM ring (one bank per startup group; 8-bank cap)

F32 = mybir.dt.float32
BF16 = mybir.dt.bfloat16
BF = ml_dtypes.bfloat16

_NC = None


def _emit(nc, xs, w1s, w3s, w2s, b1, b3, b2r, out):
    add = mybir.AluOpType.add
    mult = mybir.AluOpType.mult
    silu = mybir.ActivationFunctionType.Silu

    xs_r = xs.ap().rearrange("p (o t) -> p o t", o=DO)  # [P, DO, T]
    w1s_r = w1s.ap().rearrange("p (ft x) -> p ft x", ft=FO)  # [P, FO, DO*P]
    w3s_r = w3s.ap().rearrange("p (ft x) -> p ft x", ft=FO)
    w2s_r = w2s.ap().rearrange("p (db x) -> p db x", db=NDB)  # [P, NDB, FO*DBW]
    b1_r = b1.ap().rearrange("(o p) -> p o", p=P)  # [P, FO]
    b3_r = b3.ap().rearrange("(o p) -> p o", p=P)
    out_r = out.ap().rearrange("(o p) d -> p o d", p=P)  # [P, TT, D]

    with tile.TileContext(nc) as tc:
        with (
            tc.tile_pool(name="persist", bufs=1) as persist,
            tc.tile_pool(name="wsp", bufs=1) as wsp,
            tc.tile_pool(name="evp", bufs=1) as evp,
            tc.tile_pool(name="psp", bufs=1, space="PSUM") as psp,
        ):
            gt = persist.tile([P, FO, T], BF16, tag="gt")
            b2s = persist.tile([P, D], F32, tag="b2s")
            b1s = persist.tile([P, FO], F32, tag="b1s")
            b3s = persist.tile([P, FO], F32, tag="b3s")
            scr = persist.tile([P, 2 * P], BF16, tag="scr")
            xch = [
                persist.tile([P, XCW, T], BF16, tag=f"xc{c}", name=f"xc{c}")
                for c in range(XC)
            ]

            # ---- HAM warmup: the framework preamble pins the PE idle until
            # ~7us and the first real matmul can't start before its DMAs land
            # (~11.5us). Fill that window with zero matmuls on a DVE-memset
            # scratch tile so the HAM clock gate is already at 8/8 when the
            # real stream begins (cold MMs run at 1.2 GHz otherwise), and the
            # 3.4us busy-window has been paid off the critical path.
            nc.vector.memset(scr[:], 0.0)
            ps_warm = psp.tile([P, T], F32, tag="ps1", bufs=PS1B, name="ps_warm")
            NWARM = 24
            for i in range(NWARM):
                nc.tensor.matmul(
                    ps_warm[:, :P],
                    scr[:, :P],
                    scr[:, P:],
                    start=(i == 0),
                    stop=(i == NWARM - 1),
                )

            # ---- phase-1 startup DMA stream: x chunks + ft0/ft1 w quarters,
            # interleaved so the PE can chase the stream chunk by chunk. The
            # wq ring is deep (12 = 3 chunks of lookahead) so ring-gated
            # quarters can't convoy-block later x chunks in queue order.
            # Fixed queue roles during startup: all x on sync (7.34 MB, fully
            # resident by ~28us), all 32 w-quarters on scalar in exact
            # PE-consumption order — each queue's delivery order matches the
            # PE's need order, so neither can starve the other's stream.
            wq = {}  # (wi, ft, c) -> quarter tile [P, XCW*P]
            # (wi, ft) startup group order; 2*NPRE groups interleaved with the
            # x stream so PE demand (~252 GB/s at 6 groups) matches the
            # ~260 GB/s the two queues actually deliver during the x load —
            # at 4 groups demand was 304 GB/s and the PE starved + HAM
            # oscillated K=8/8 -> 4/8 five times in the first 90us.
            pre = [(wi, ft) for ft in range(NPRE) for wi in range(2)]
            for c in range(XC):
                if c < 2:
                    # per-subtile slivers: each matmul j is gated only on its
                    # own 131 KB sliver, so the PE chases the x stream at DMA
                    # granularity instead of stalling on whole 0.92 MB chunks
                    for j in range(XCW):
                        nc.sync.dma_start(
                            xch[c][:, j, :], xs_r[:, c * XCW + j, :]
                        )
                else:
                    nc.sync.dma_start(
                        xch[c][:], xs_r[:, c * XCW : (c + 1) * XCW, :]
                    )
                for wi, ft in pre:
                    wsrc = w1s_r if wi == 0 else w3s_r
                    q = wsp.tile([P, XCW * P], BF16, tag="wq", bufs=12, name="wq")
                    # third queue for the extra NPRE=3 quarter stream: the
                    # two-queue startup topped out ~260 GB/s; gpsimd's queue
                    # adds headroom (combined 3-queue peak observed >360)
                    qeng = nc.gpsimd if ft == NPRE - 1 else nc.scalar
                    qeng.dma_start(
                        q[:], wsrc[:, ft, c * XCW * P : (c + 1) * XCW * P]
                    )
                    wq[(wi, ft, c)] = q
            # biases are tiny (8 KB); ride the otherwise-idle gpsimd queue so
            # they never contend with the x/weight streams. Consumers (the
            # part-A evictions) are emitted below, so Tile sequences them.
            nc.gpsimd.dma_start(b1s[:], b1_r)
            nc.gpsimd.dma_start(b3s[:], b3_r)

            # ---- phase-1 startup matmuls: 4 groups interleaved chunk-wise.
            ps_pre = {}
            for wi, ft in pre:
                ps_pre[(wi, ft)] = psp.tile(
                    [P, T], F32, tag="ps1", bufs=PS1B, name="ps1"
                )
            for c in range(XC):
                for wi, ft in pre:
                    q = wq[(wi, ft, c)]
                    for j in range(XCW):
                        nc.tensor.matmul(
                            ps_pre[(wi, ft)][:],
                            q[:, j * P : (j + 1) * P],
                            xch[c][:, j, :],
                            start=(c == 0 and j == 0),
                            stop=(c == XC - 1 and j == XCW - 1),
                        )

            s1_cur = {}  # ft -> s1 tile

            def evict_p1(wi, ft, ps):
                if wi == 0:
                    s1 = evp.tile([P, T], F32, tag="s1", bufs=2, name="s1")
                    nc.scalar.activation(
                        s1[:], ps[:], silu, bias=b1s[:, ft : ft + 1]
                    )
                    s1_cur[ft] = s1
                else:
                    nc.vector.scalar_tensor_tensor(
                        out=gt[:, ft, :],
                        in0=ps[:],
                        scalar=b3s[:, ft : ft + 1],
                        in1=s1_cur[ft][:],
                        op0=add,
                        op1=mult,
                    )

            for wi, ft in pre:
                evict_p1(wi, ft, ps_pre[(wi, ft)])

            # ---- phase-1 steady state: ft 2..15, full 56-deep groups.
            # ft2/ft3 halves ride directly behind the part-A stream so they
            # land before the PE drains part A; biases after them; b2s later.
            halves = {}
            for ft in range(NPRE, FO):
                for wi in range(2):
                    wsrc = w1s_r if wi == 0 else w3s_r
                    hA = wsp.tile([P, HW_ * P], BF16, tag="wh", bufs=4, name="whA")
                    hB = wsp.tile([P, HW_ * P], BF16, tag="wh", bufs=4, name="whB")
                    nc.sync.dma_start(hA[:], wsrc[:, ft, : HW_ * P])
                    nc.scalar.dma_start(hB[:], wsrc[:, ft, HW_ * P :])
                    halves[(wi, ft)] = (hA, hB)

            for ft in range(NPRE, FO):
                for wi in range(2):
                    hA, hB = halves[(wi, ft)]
                    ps = psp.tile([P, T], F32, tag="ps1", bufs=PS1B, name="ps1")
                    for o in range(DO):
                        if o < HW_:
                            lhsT = hA[:, o * P : (o + 1) * P]
                        else:
                            lhsT = hB[:, (o - HW_) * P : (o - HW_ + 1) * P]
                        nc.tensor.matmul(
                            ps[:],
                            lhsT,
                            xch[o // XCW][:, o % XCW, :],
                            start=(o == 0),
                            stop=(o == DO - 1),
                        )
                    evict_p1(wi, ft, ps)
                # b2 broadcast (3.67 MB) in two 256 KB chunks per late ft
                # group on the scalar queue: the trigger instructions sit
                # behind the silu ACT in the scalar engine's FIFO, so the
                # transfers are time-gated by PE progress and land in the
                # second half of phase 1 where HBM has ~200 GB/s of slack
                # (injecting it into the weight queues at ft==5 caused a
                # 12.6us PE stall + a 20us HAM re-throttle in v2).
                if 8 <= ft <= 14:
                    for cc in (2 * (ft - 8), 2 * (ft - 8) + 1):
                        nc.scalar.dma_start(
                            b2s[:, cc * 512 : (cc + 1) * 512],
                            b2r.ap()[:, cc * 512 : (cc + 1) * 512],
                        )

            # ---------------- phase 2 ----------------
            # BOTH w2 halves ride sync: one queue pumps each ring-gated half
            # at its full ~204 GB/s (per-db delivery 20.6us < 27.6us
            # consumption). Splitting them sync/scalar (tried in v4) made the
            # two ring-gated streams alternate at ~105 GB/s each against the
            # per-core HBM cap, landing each db's hB ~13us late -> one PE
            # stall per db at fo=8 + a HAM re-throttle. Output DMAs ride the
            # gpsimd queue (their triggers carry semaphore waits on the DVE
            # adds) so a ring-gated weight DMA can never head-of-line-block
            # them.
            w2h = {}
            for db in range(NDB):
                hA = wsp.tile([P, HFO * DBW], BF16, tag="w2h", bufs=3, name="w2hA")
                hB = wsp.tile([P, HFO * DBW], BF16, tag="w2h", bufs=3, name="w2hB")
                nc.sync.dma_start(hA[:], w2s_r[:, db, : HFO * DBW])
                nc.sync.dma_start(hB[:], w2s_r[:, db, HFO * DBW :])
                w2h[db] = (hA, hB)

            # per-512 PSUM groups (2 banks total) leave room for the 6-bank
            # phase-1 ring; evictions + output DMAs go per 512 columns. All
            # output triggers ride gpsimd — sync/scalar carry the ring-gated
            # w2 triggers whose semaphore waits would head-of-line-block any
            # output trigger emitted behind them. Exception: the very last
            # transfer goes to sync (its w2 ring waits have long resolved by
            # then) so the final two 256 KB transfers overlap.
            for db in range(NDB):
                hA, hB = w2h[db]
                for ts in range(TT):
                    for half in range(2):
                        off = half * 512
                        ps2 = psp.tile([P, 512], F32, tag="ps2", bufs=2, name="ps2")
                        lhsT = gt[:, :, ts * P : (ts + 1) * P]
                        for fo in range(FO):
                            wh = hA if fo < HFO else hB
                            base = (fo % HFO) * DBW + off
                            nc.tensor.matmul(
                                ps2[:],
                                lhsT[:, fo, :],
                                wh[:, base : base + 512],
                                start=(fo == 0),
                                stop=(fo == FO - 1),
                            )
                        ost = evp.tile([P, 512], F32, tag="ost", bufs=2, name="ost")
                        dlo = db * DBW + off
                        nc.vector.tensor_add(ost[:], ps2[:], b2s[:, dlo : dlo + 512])
                        last = db == NDB - 1 and ts == TT - 1 and half == 1
                        q = nc.scalar if last else nc.gpsimd
                        q.dma_start(out_r[:, ts, dlo : dlo + 512], ost[:])


def build():
    global _NC
    if _NC is not None:
        return _NC
    nc = bacc.Bacc("TRN2", target_bir_lowering=False, debug=False, num_devices=E)
    xs = nc.dram_tensor("xs", [P, DO * T], BF16, kind="ExternalInput")
    w1s = nc.dram_tensor("w1s", [P, FO * DO * P], BF16, kind="ExternalInput")
    w3s = nc.dram_tensor("w3s", [P, FO * DO * P], BF16, kind="ExternalInput")
    w2s = nc.dram_tensor("w2s", [P, NDB * FO * DBW], BF16, kind="ExternalInput")
    b1 = nc.dram_tensor("b1", [F], F32, kind="ExternalInput")
    b3 = nc.dram_tensor("b3", [F], F32, kind="ExternalInput")
    b2r = nc.dram_tensor("b2r", [P, D], F32, kind="ExternalInput")
    out = nc.dram_tensor("out", [T, D], F32, kind="ExternalOutput")
    _emit(nc, xs, w1s, w3s, w2s, b1, b3, b2r, out)
    nc.compile()
    _NC = nc
    return nc


def make_in_maps(x, w1, b1, w3, b3, w2, b2):
    x = np.asarray(x, dtype=np.float32)
    w1 = np.asarray(w1, dtype=np.float32)
    b1 = np.asarray(b1, dtype=np.float32)
    w3 = np.asarray(w3, dtype=np.float32)
    b3 = np.asarray(b3, dtype=np.float32)
    w2 = np.asarray(w2, dtype=np.float32)
    b2 = np.asarray(b2, dtype=np.float32)
    in_maps = []
    for e in range(E):
        # xs[p][o][t] = x[e][t, o*128+p]
        xs = x[e].reshape(T, DO, P).transpose(2, 1, 0).reshape(P, -1).astype(BF)
        # w1s[p][ft][o][fl] = w1[e][ft*128+fl, o*128+p]
        w1s = (
            w1[e].reshape(FO, P, DO, P).transpose(3, 0, 2, 1).reshape(P, -1).astype(BF)
        )
        w3s = (
            w3[e].reshape(FO, P, DO, P).transpose(3, 0, 2, 1).reshape(P, -1).astype(BF)
        )
        # w2s[p][db][fo][dw] = w2[e][db*1024+dw, fo*128+p]
        w2s = (
            w2[e]
            .reshape(NDB, DBW, FO, P)
            .transpose(3, 0, 2, 1)
            .reshape(P, -1)
            .astype(BF)
        )
        in_maps.append(
            {
                "xs": xs,
                "w1s": w1s,
                "w3s": w3s,
                "w2s": w2s,
                "b1": b1[e],
                "b3": b3[e],
                "b2r": np.ascontiguousarray(np.broadcast_to(b2[e], (P, D))),
            }
        )
    return in_maps


def run(x, w1, b1, w3, b3, w2, b2, **spmd_kwargs):
    nc = build()
    in_maps = make_in_maps(x, w1, b1, w3, b3, w2, b2)
    res = run_bass_kernel_spmd(nc, in_maps, core_ids=list(range(E)), **spmd_kwargs)
    out = np.stack([res.results[e]["out"] for e in range(E)], axis=0)
    return out, res


def kernel(x, w1, b1, w3, b3, w2, b2):
    out, _ = run(x, w1, b1, w3, b3, w2, b2)
    return out



# revision 26
# speedup vs baseline: 1.1162x; 1.0669x over previous
"""Expert-parallel batched SwiGLU FFN for Trainium2 (8 NeuronCores, Bass/Tile).

Problem: out[e] = (silu(x[e] @ w1[e].T + b1[e]) * (x[e] @ w3[e].T + b3[e])) @ w2[e].T + b2[e]
with E=8, T=512, D_IN=7168, D_INT=2048, fp32 reference.

Sharding: expert-parallel - core e owns expert e end-to-end, no communication.

v2 strategy (from the v1 trace): v1 ran fp32r everywhere, putting HBM traffic
(209 MB/core = 584 us at 358 GB/s) right on top of the PE roofline (2688
512-col matmuls = 580 us), so every DMA hiccup was a PE stall (39 us of gaps +
7 HAM re-throttles). v2 stages x/w1/w3/w2 as bf16 (halves DMA to ~114 MB =
~320 us, same PE rate, rel err ~5e-4 -> ~4e-3, well under the 2e-2 gate) so the
PE is the sole critical engine.

v3 (from the v2 trace, 658us run: steady state is a clean 216 ns/MM; losses
were 12.6us stall at ft~5 + 20us HAM K=4/8 re-throttle it triggered (the 3.67MB
b2 broadcast injected into both weight queues), ~9us of startup gaps, ~12us
tail, plus ~27us of uncontrollable GPIO power-throttle):
  - b2 broadcast chunked onto the scalar queue behind phase-1 silu ACTs
    (time-gated by PE progress, lands in the HBM-slack half of phase 1)
  - b1/b3 on the idle gpsimd queue
  - HAM warmup: 24 zero matmuls during the preamble/first-DMA dead window
  - startup x chunks 0-1 split per-subtile for DMA-granularity chasing
  - phase 2: w2 halves split sync/scalar, outputs on the gpsimd queue

v4 (from the v3 trace, 636us run: tail is near-floor at ~7us; the loss is
~25us of startup gaps + 5 HAM K=4/8 oscillations in the first 90us — with 4
interleaved startup groups the PE demands ~304 GB/s while the two queues
deliver ~260 GB/s during the x load):
  - NPRE=3: 6 interleaved startup groups drop demand to ~252 GB/s (matched)
  - PSUM rebalance: phase-1 ring 6 banks, phase-2 per-512 groups (2 banks)

Layouts are host-swizzled so every DMA is per-partition contiguous:
  xs  [128][o][t]        o = d//128 (56), partition = d%128
  w1s/w3s [128][ft][o][fl]  ft = f//128 (16)
  w2s [128][db][fo][dw]  db = d//1024 (7), fo = f//128, dw = d%1024

Per-core schedule:
  phase 1: x fully SBUF-resident (56 KiB/part bf16). For each (ft, w in
    {w1,w3}): one 56-deep PSUM accumulation group (stationary = w tile
    [128d,128f], moving = x [128d,512t]); silu+bias straight off PSUM on
    ScalarE, then gt[ft] = (h3+b3)*s1 on DVE, written bf16.
    Startup: the ft0+ft1 group quadruple is interleaved chunk-by-chunk with
    the x DMA stream (8 chunks of 7 d-subtiles) so the PE starts ~3.5 us in
    instead of waiting ~20 us for all of x.
  phase 2: out[t,d] = sum_f gt[f,t]*w2[f,d]. gt tiles stationary, w2 moving,
    16-deep PSUM groups over fo into [128,1024] (2-bank) tiles per (db,ts);
    DVE adds b2 (host-broadcast) and output DMAs stream out per (db,ts).
    w2 blocks ride queue-S only, output DMAs queue-A only, so a ring-gated
    weight DMA can never head-of-line-block an output transfer.
"""

import numpy as np
import ml_dtypes

import concourse.bacc as bacc
import concourse.mybir as mybir
import concourse.tile as tile
from concourse.bass_utils import run_bass_kernel_spmd

# Problem shape (hardcoded per contest contract).
E = 8
T = 512
D = 7168
F = 2048
P = 128

DO = D // P  # 56 d-subtiles
FO = F // P  # 16 f-subtiles
TT = T // P  # 4 t-subtiles

XC = 8  # x chunks (phase-1 startup granularity)
XCW = DO // XC  # 7 d-subtiles per x chunk
NPRE = 3  # ft groups interleaved with the x stream at startup
HW_ = 28  # phase-1 steady-state weight half-block width (d-subtiles)
DBW = 1024  # phase-2 block width (d)
NDB = D // DBW  # 7 blocks
HFO = FO // 2  # 8: phase-2 w2 half block (fo)

PS1B = 2 * NPRE  # phase-1 PSUM ring (one bank per startup group; 8-bank cap)

F32 = mybir.dt.float32
BF16 = mybir.dt.bfloat16
BF = ml_dtypes.bfloat16

_NC = None


def _emit(nc, xs, w1s, w3s, w2s, b1, b3, b2r, out):
    add = mybir.AluOpType.add
    mult = mybir.AluOpType.mult
    silu = mybir.ActivationFunctionType.Silu

    xs_r = xs.ap().rearrange("p (o t) -> p o t", o=DO)  # [P, DO, T]
    w1s_r = w1s.ap().rearrange("p (ft x) -> p ft x", ft=FO)  # [P, FO, DO*P]
    w3s_r = w3s.ap().rearrange("p (ft x) -> p ft x", ft=FO)
    w2s_r = w2s.ap().rearrange("p (db x) -> p db x", db=NDB)  # [P, NDB, FO*DBW]
    b1_r = b1.ap().rearrange("(o p) -> p o", p=P)  # [P, FO]
    b3_r = b3.ap().rearrange("(o p) -> p o", p=P)
    out_r = out.ap().rearrange("(o p) d -> p o d", p=P)  # [P, TT, D]

    with tile.TileContext(nc) as tc:
        with (
            tc.tile_pool(name="persist", bufs=1) as persist,
            tc.tile_pool(name="wsp", bufs=1) as wsp,
            tc.tile_pool(name="evp", bufs=1) as evp,
            tc.tile_pool(name="psp", bufs=1, space="PSUM") as psp,
        ):
            gt = persist.tile([P, FO, T], BF16, tag="gt")
            b2s = persist.tile([P, D], F32, tag="b2s")
            b1s = persist.tile([P, FO], F32, tag="b1s")
            b3s = persist.tile([P, FO], F32, tag="b3s")
            scr = persist.tile([P, 2 * P], BF16, tag="scr")
            xch = [
                persist.tile([P, XCW, T], BF16, tag=f"xc{c}", name=f"xc{c}")
                for c in range(XC)
            ]

            # ---- HAM warmup: the framework preamble pins the PE idle until
            # ~7us and the first real matmul can't start before its DMAs land
            # (~11.5us). Fill that window with zero matmuls on a DVE-memset
            # scratch tile so the HAM clock gate is already at 8/8 when the
            # real stream begins (cold MMs run at 1.2 GHz otherwise), and the
            # 3.4us busy-window has been paid off the critical path.
            nc.vector.memset(scr[:], 0.0)
            ps_warm = psp.tile([P, T], F32, tag="ps1", bufs=PS1B, name="ps_warm")
            NWARM = 24
            for i in range(NWARM):
                nc.tensor.matmul(
                    ps_warm[:, :P],
                    scr[:, :P],
                    scr[:, P:],
                    start=(i == 0),
                    stop=(i == NWARM - 1),
                )

            # ---- phase-1 startup DMA stream: x chunks + ft0/ft1 w quarters,
            # interleaved so the PE can chase the stream chunk by chunk. The
            # wq ring is deep (12 = 3 chunks of lookahead) so ring-gated
            # quarters can't convoy-block later x chunks in queue order.
            # Fixed queue roles during startup: all x on sync (7.34 MB, fully
            # resident by ~28us), all 32 w-quarters on scalar in exact
            # PE-consumption order — each queue's delivery order matches the
            # PE's need order, so neither can starve the other's stream.
            wq = {}  # (wi, ft, c) -> quarter tile [P, XCW*P]
            # (wi, ft) startup group order; 2*NPRE groups interleaved with the
            # x stream so PE demand (~252 GB/s at 6 groups) matches the
            # ~260 GB/s the two queues actually deliver during the x load —
            # at 4 groups demand was 304 GB/s and the PE starved + HAM
            # oscillated K=8/8 -> 4/8 five times in the first 90us.
            pre = [(wi, ft) for ft in range(NPRE) for wi in range(2)]
            for c in range(XC):
                if c < 2:
                    # per-subtile slivers: each matmul j is gated only on its
                    # own 131 KB sliver, so the PE chases the x stream at DMA
                    # granularity instead of stalling on whole 0.92 MB chunks
                    for j in range(XCW):
                        nc.sync.dma_start(
                            xch[c][:, j, :], xs_r[:, c * XCW + j, :]
                        )
                else:
                    nc.sync.dma_start(
                        xch[c][:], xs_r[:, c * XCW : (c + 1) * XCW, :]
                    )
                for wi, ft in pre:
                    wsrc = w1s_r if wi == 0 else w3s_r
                    q = wsp.tile([P, XCW * P], BF16, tag="wq", bufs=12, name="wq")
                    # queue balance: scalar carries 4 quarter streams
                    # (101 GB/s), sync carries x + the ft2 streams (152 GB/s)
                    # in PE-consumption order (x first, ft2 last). gpsimd's
                    # queue measured only ~80 GB/s - too slow for this.
                    qeng = nc.sync if ft == NPRE - 1 else nc.scalar
                    qeng.dma_start(
                        q[:], wsrc[:, ft, c * XCW * P : (c + 1) * XCW * P]
                    )
                    wq[(wi, ft, c)] = q
            # biases are tiny (8 KB); ride the otherwise-idle gpsimd queue so
            # they never contend with the x/weight streams. Consumers (the
            # part-A evictions) are emitted below, so Tile sequences them.
            nc.gpsimd.dma_start(b1s[:], b1_r)
            nc.gpsimd.dma_start(b3s[:], b3_r)

            # ---- phase-1 startup matmuls: 4 groups interleaved chunk-wise.
            ps_pre = {}
            for wi, ft in pre:
                ps_pre[(wi, ft)] = psp.tile(
                    [P, T], F32, tag="ps1", bufs=PS1B, name="ps1"
                )
            for c in range(XC):
                for wi, ft in pre:
                    q = wq[(wi, ft, c)]
                    for j in range(XCW):
                        nc.tensor.matmul(
                            ps_pre[(wi, ft)][:],
                            q[:, j * P : (j + 1) * P],
                            xch[c][:, j, :],
                            start=(c == 0 and j == 0),
                            stop=(c == XC - 1 and j == XCW - 1),
                        )

            s1_cur = {}  # ft -> s1 tile

            def evict_p1(wi, ft, ps):
                if wi == 0:
                    s1 = evp.tile([P, T], F32, tag="s1", bufs=2, name="s1")
                    nc.scalar.activation(
                        s1[:], ps[:], silu, bias=b1s[:, ft : ft + 1]
                    )
                    s1_cur[ft] = s1
                else:
                    nc.vector.scalar_tensor_tensor(
                        out=gt[:, ft, :],
                        in0=ps[:],
                        scalar=b3s[:, ft : ft + 1],
                        in1=s1_cur[ft][:],
                        op0=add,
                        op1=mult,
                    )

            for wi, ft in pre:
                evict_p1(wi, ft, ps_pre[(wi, ft)])

            # ---- phase-1 steady state: ft 2..15, full 56-deep groups.
            # ft2/ft3 halves ride directly behind the part-A stream so they
            # land before the PE drains part A; biases after them; b2s later.
            halves = {}
            for ft in range(NPRE, FO):
                for wi in range(2):
                    wsrc = w1s_r if wi == 0 else w3s_r
                    hA = wsp.tile([P, HW_ * P], BF16, tag="wh", bufs=4, name="whA")
                    hB = wsp.tile([P, HW_ * P], BF16, tag="wh", bufs=4, name="whB")
                    nc.sync.dma_start(hA[:], wsrc[:, ft, : HW_ * P])
                    nc.scalar.dma_start(hB[:], wsrc[:, ft, HW_ * P :])
                    halves[(wi, ft)] = (hA, hB)

            for ft in range(NPRE, FO):
                for wi in range(2):
                    hA, hB = halves[(wi, ft)]
                    ps = psp.tile([P, T], F32, tag="ps1", bufs=PS1B, name="ps1")
                    for o in range(DO):
                        if o < HW_:
                            lhsT = hA[:, o * P : (o + 1) * P]
                        else:
                            lhsT = hB[:, (o - HW_) * P : (o - HW_ + 1) * P]
                        nc.tensor.matmul(
                            ps[:],
                            lhsT,
                            xch[o // XCW][:, o % XCW, :],
                            start=(o == 0),
                            stop=(o == DO - 1),
                        )
                    evict_p1(wi, ft, ps)
                # b2 broadcast (3.67 MB) in two 256 KB chunks per late ft
                # group on the scalar queue: the trigger instructions sit
                # behind the silu ACT in the scalar engine's FIFO, so the
                # transfers are time-gated by PE progress and land in the
                # second half of phase 1 where HBM has ~200 GB/s of slack
                # (injecting it into the weight queues at ft==5 caused a
                # 12.6us PE stall + a 20us HAM re-throttle in v2).
                if 8 <= ft <= 14:
                    for cc in (2 * (ft - 8), 2 * (ft - 8) + 1):
                        nc.scalar.dma_start(
                            b2s[:, cc * 512 : (cc + 1) * 512],
                            b2r.ap()[:, cc * 512 : (cc + 1) * 512],
                        )

            # ---------------- phase 2 ----------------
            # BOTH w2 halves ride sync: one queue pumps each ring-gated half
            # at its full ~204 GB/s (per-db delivery 20.6us < 27.6us
            # consumption). Splitting them sync/scalar (tried in v4) made the
            # two ring-gated streams alternate at ~105 GB/s each against the
            # per-core HBM cap, landing each db's hB ~13us late -> one PE
            # stall per db at fo=8 + a HAM re-throttle.
            # The halves split the db block by d (dh), not by fo, and `half`
            # is the OUTER compute loop: hA's last consumer is then half a db
            # (13.8us) before hB's, so the two ring-recycle waits resolve
            # staggered and the sync queue streams continuously (fo-split
            # halves, tried in v5, had both last consumers at the db end ->
            # both next-halves fired together and late -> one stall per db).
            # Output DMAs ride the gpsimd queue (their triggers carry
            # semaphore waits on the DVE adds) so a ring-gated weight DMA
            # can never head-of-line-block them.
            w2h = {}
            for db in range(NDB):
                hA = wsp.tile([P, HFO * DBW], BF16, tag="w2h", bufs=3, name="w2hA")
                hB = wsp.tile([P, HFO * DBW], BF16, tag="w2h", bufs=3, name="w2hB")
                nc.sync.dma_start(hA[:], w2s_r[:, db, : HFO * DBW])
                nc.sync.dma_start(hB[:], w2s_r[:, db, HFO * DBW :])
                w2h[db] = (hA, hB)

            # per-512 PSUM groups (2 banks total) leave room for the 6-bank
            # phase-1 ring; evictions + output DMAs go per 512 columns.
            for db in range(NDB):
                hA, hB = w2h[db]
                for half in range(2):
                    wh = hA if half == 0 else hB
                    dlo = db * DBW + half * 512
                    for ts in range(TT):
                        ps2 = psp.tile([P, 512], F32, tag="ps2", bufs=2, name="ps2")
                        for fo in range(FO):
                            nc.tensor.matmul(
                                ps2[:],
                                gt[:, fo, ts * P : (ts + 1) * P],
                                wh[:, fo * 512 : (fo + 1) * 512],
                                start=(fo == 0),
                                stop=(fo == FO - 1),
                            )
                        ost = evp.tile([P, 512], F32, tag="ost", bufs=2, name="ost")
                        nc.vector.tensor_add(ost[:], ps2[:], b2s[:, dlo : dlo + 512])
                        last = db == NDB - 1 and half == 1 and ts == TT - 1
                        q = nc.scalar if last else nc.gpsimd
                        q.dma_start(out_r[:, ts, dlo : dlo + 512], ost[:])


def build():
    global _NC
    if _NC is not None:
        return _NC
    nc = bacc.Bacc("TRN2", target_bir_lowering=False, debug=False, num_devices=E)
    xs = nc.dram_tensor("xs", [P, DO * T], BF16, kind="ExternalInput")
    w1s = nc.dram_tensor("w1s", [P, FO * DO * P], BF16, kind="ExternalInput")
    w3s = nc.dram_tensor("w3s", [P, FO * DO * P], BF16, kind="ExternalInput")
    w2s = nc.dram_tensor("w2s", [P, NDB * FO * DBW], BF16, kind="ExternalInput")
    b1 = nc.dram_tensor("b1", [F], F32, kind="ExternalInput")
    b3 = nc.dram_tensor("b3", [F], F32, kind="ExternalInput")
    b2r = nc.dram_tensor("b2r", [P, D], F32, kind="ExternalInput")
    out = nc.dram_tensor("out", [T, D], F32, kind="ExternalOutput")
    _emit(nc, xs, w1s, w3s, w2s, b1, b3, b2r, out)
    nc.compile()
    _NC = nc
    return nc


def make_in_maps(x, w1, b1, w3, b3, w2, b2):
    x = np.asarray(x, dtype=np.float32)
    w1 = np.asarray(w1, dtype=np.float32)
    b1 = np.asarray(b1, dtype=np.float32)
    w3 = np.asarray(w3, dtype=np.float32)
    b3 = np.asarray(b3, dtype=np.float32)
    w2 = np.asarray(w2, dtype=np.float32)
    b2 = np.asarray(b2, dtype=np.float32)
    in_maps = []
    for e in range(E):
        # xs[p][o][t] = x[e][t, o*128+p]
        xs = x[e].reshape(T, DO, P).transpose(2, 1, 0).reshape(P, -1).astype(BF)
        # w1s[p][ft][o][fl] = w1[e][ft*128+fl, o*128+p]
        w1s = (
            w1[e].reshape(FO, P, DO, P).transpose(3, 0, 2, 1).reshape(P, -1).astype(BF)
        )
        w3s = (
            w3[e].reshape(FO, P, DO, P).transpose(3, 0, 2, 1).reshape(P, -1).astype(BF)
        )
        # w2s[p][db][dh][fo][dw] = w2[e][db*1024 + dh*512 + dw, fo*128+p]
        # (dh = d-half of the 1024 block: phase-2 `half` loop consumes one
        #  dh-half tile at a time, staggering the two ring-recycle waits)
        w2s = (
            w2[e]
            .reshape(NDB, 2, 512, FO, P)
            .transpose(4, 0, 1, 3, 2)
            .reshape(P, -1)
            .astype(BF)
        )
        in_maps.append(
            {
                "xs": xs,
                "w1s": w1s,
                "w3s": w3s,
                "w2s": w2s,
                "b1": b1[e],
                "b3": b3[e],
                "b2r": np.ascontiguousarray(np.broadcast_to(b2[e], (P, D))),
            }
        )
    return in_maps


def run(x, w1, b1, w3, b3, w2, b2, **spmd_kwargs):
    nc = build()
    in_maps = make_in_maps(x, w1, b1, w3, b3, w2, b2)
    res = run_bass_kernel_spmd(nc, in_maps, core_ids=list(range(E)), **spmd_kwargs)
    out = np.stack([res.results[e]["out"] for e in range(E)], axis=0)
    return out, res


def kernel(x, w1, b1, w3, b3, w2, b2):
    out, _ = run(x, w1, b1, w3, b3, w2, b2)
    return out



# revision 29
# speedup vs baseline: 1.1263x; 1.0090x over previous
"""Expert-parallel batched SwiGLU FFN for Trainium2 (8 NeuronCores, Bass/Tile).

Problem: out[e] = (silu(x[e] @ w1[e].T + b1[e]) * (x[e] @ w3[e].T + b3[e])) @ w2[e].T + b2[e]
with E=8, T=512, D_IN=7168, D_INT=2048, fp32 reference.

Sharding: expert-parallel - core e owns expert e end-to-end, no communication.

v2 strategy (from the v1 trace): v1 ran fp32r everywhere, putting HBM traffic
(209 MB/core = 584 us at 358 GB/s) right on top of the PE roofline (2688
512-col matmuls = 580 us), so every DMA hiccup was a PE stall (39 us of gaps +
7 HAM re-throttles). v2 stages x/w1/w3/w2 as bf16 (halves DMA to ~114 MB =
~320 us, same PE rate, rel err ~5e-4 -> ~4e-3, well under the 2e-2 gate) so the
PE is the sole critical engine.

v3 (from the v2 trace, 658us run: steady state is a clean 216 ns/MM; losses
were 12.6us stall at ft~5 + 20us HAM K=4/8 re-throttle it triggered (the 3.67MB
b2 broadcast injected into both weight queues), ~9us of startup gaps, ~12us
tail, plus ~27us of uncontrollable GPIO power-throttle):
  - b2 broadcast chunked onto the scalar queue behind phase-1 silu ACTs
    (time-gated by PE progress, lands in the HBM-slack half of phase 1)
  - b1/b3 on the idle gpsimd queue
  - HAM warmup: 24 zero matmuls during the preamble/first-DMA dead window
  - startup x chunks 0-1 split per-subtile for DMA-granularity chasing
  - phase 2: w2 halves split sync/scalar, outputs on the gpsimd queue

v4 (from the v3 trace, 636us run: tail is near-floor at ~7us; the loss is
~25us of startup gaps + 5 HAM K=4/8 oscillations in the first 90us — with 4
interleaved startup groups the PE demands ~304 GB/s while the two queues
deliver ~260 GB/s during the x load):
  - NPRE=3: 6 interleaved startup groups drop demand to ~252 GB/s (matched)
  - PSUM rebalance: phase-1 ring 6 banks, phase-2 per-512 groups (2 banks)

Layouts are host-swizzled so every DMA is per-partition contiguous:
  xs  [128][o][t]        o = d//128 (56), partition = d%128
  w1s/w3s [128][ft][o][fl]  ft = f//128 (16)
  w2s [128][db][fo][dw]  db = d//1024 (7), fo = f//128, dw = d%1024

Per-core schedule:
  phase 1: x fully SBUF-resident (56 KiB/part bf16). For each (ft, w in
    {w1,w3}): one 56-deep PSUM accumulation group (stationary = w tile
    [128d,128f], moving = x [128d,512t]); silu+bias straight off PSUM on
    ScalarE, then gt[ft] = (h3+b3)*s1 on DVE, written bf16.
    Startup: the ft0+ft1 group quadruple is interleaved chunk-by-chunk with
    the x DMA stream (8 chunks of 7 d-subtiles) so the PE starts ~3.5 us in
    instead of waiting ~20 us for all of x.
  phase 2: out[t,d] = sum_f gt[f,t]*w2[f,d]. gt tiles stationary, w2 moving,
    16-deep PSUM groups over fo into [128,1024] (2-bank) tiles per (db,ts);
    DVE adds b2 (host-broadcast) and output DMAs stream out per (db,ts).
    w2 blocks ride queue-S only, output DMAs queue-A only, so a ring-gated
    weight DMA can never head-of-line-block an output transfer.
"""

import numpy as np
import ml_dtypes

import concourse.bacc as bacc
import concourse.mybir as mybir
import concourse.tile as tile
from concourse.bass_utils import run_bass_kernel_spmd

# Problem shape (hardcoded per contest contract).
E = 8
T = 512
D = 7168
F = 2048
P = 128

DO = D // P  # 56 d-subtiles
FO = F // P  # 16 f-subtiles
TT = T // P  # 4 t-subtiles

XC = 8  # x chunks (phase-1 startup granularity)
XCW = DO // XC  # 7 d-subtiles per x chunk
NPRE = 3  # ft groups interleaved with the x stream at startup
HW_ = 28  # phase-1 steady-state weight half-block width (d-subtiles)
DBW = 1024  # phase-2 block width (d)
NDB = D // DBW  # 7 blocks
HFO = FO // 2  # 8: phase-2 w2 half block (fo)

PS1B = 2 * NPRE  # phase-1 PSUM ring (one bank per startup group; 8-bank cap)

F32 = mybir.dt.float32
BF16 = mybir.dt.bfloat16
BF = ml_dtypes.bfloat16

_NC = None


def _emit(nc, xs, w1s, w3s, w2s, b1, b3, b2r, out):
    add = mybir.AluOpType.add
    mult = mybir.AluOpType.mult
    silu = mybir.ActivationFunctionType.Silu

    xs_r = xs.ap().rearrange("p (o t) -> p o t", o=DO)  # [P, DO, T]
    w1s_r = w1s.ap().rearrange("p (ft x) -> p ft x", ft=FO)  # [P, FO, DO*P]
    w3s_r = w3s.ap().rearrange("p (ft x) -> p ft x", ft=FO)
    w2s_r = w2s.ap().rearrange("p (db x) -> p db x", db=NDB)  # [P, NDB, FO*DBW]
    b1_r = b1.ap().rearrange("(o p) -> p o", p=P)  # [P, FO]
    b3_r = b3.ap().rearrange("(o p) -> p o", p=P)
    out_r = out.ap().rearrange("(o p) d -> p o d", p=P)  # [P, TT, D]

    with tile.TileContext(nc) as tc:
        with (
            tc.tile_pool(name="persist", bufs=1) as persist,
            tc.tile_pool(name="wsp", bufs=1) as wsp,
            tc.tile_pool(name="evp", bufs=1) as evp,
            tc.tile_pool(name="psp", bufs=1, space="PSUM") as psp,
        ):
            gt = persist.tile([P, FO, T], BF16, tag="gt")
            b2s = persist.tile([P, D], F32, tag="b2s")
            b1s = persist.tile([P, FO], F32, tag="b1s")
            b3s = persist.tile([P, FO], F32, tag="b3s")
            scr = persist.tile([P, 2 * P], BF16, tag="scr")
            xch = [
                persist.tile([P, XCW, T], BF16, tag=f"xc{c}", name=f"xc{c}")
                for c in range(XC)
            ]

            # ---- HAM warmup: the framework preamble pins the PE idle until
            # ~7us and the first real matmul can't start before its DMAs land
            # (~11.5us). Fill that window with zero matmuls on a DVE-memset
            # scratch tile so the HAM clock gate is already at 8/8 when the
            # real stream begins (cold MMs run at 1.2 GHz otherwise), and the
            # 3.4us busy-window has been paid off the critical path.
            nc.vector.memset(scr[:], 0.0)
            ps_warm = psp.tile([P, T], F32, tag="ps1", bufs=PS1B, name="ps_warm")
            NWARM = 32
            for i in range(NWARM):
                nc.tensor.matmul(
                    ps_warm[:, :P],
                    scr[:, :P],
                    scr[:, P:],
                    start=(i == 0),
                    stop=(i == NWARM - 1),
                )

            # ---- phase-1 startup DMA stream: x chunks + ft0/ft1 w quarters,
            # interleaved so the PE can chase the stream chunk by chunk. The
            # wq ring is deep (12 = 3 chunks of lookahead) so ring-gated
            # quarters can't convoy-block later x chunks in queue order.
            # Fixed queue roles during startup: all x on sync (7.34 MB, fully
            # resident by ~28us), all 32 w-quarters on scalar in exact
            # PE-consumption order — each queue's delivery order matches the
            # PE's need order, so neither can starve the other's stream.
            wq = {}  # (wi, ft, c) -> quarter tile [P, XCW*P]
            # (wi, ft) startup group order; 2*NPRE groups interleaved with the
            # x stream so PE demand (~252 GB/s at 6 groups) matches the
            # ~260 GB/s the two queues actually deliver during the x load —
            # at 4 groups demand was 304 GB/s and the PE starved + HAM
            # oscillated K=8/8 -> 4/8 five times in the first 90us.
            pre = [(wi, ft) for ft in range(NPRE) for wi in range(2)]
            for c in range(XC):
                if c < 2:
                    # per-subtile slivers: each matmul j is gated only on its
                    # own 131 KB sliver, so the PE chases the x stream at DMA
                    # granularity instead of stalling on whole 0.92 MB chunks
                    for j in range(XCW):
                        nc.sync.dma_start(
                            xch[c][:, j, :], xs_r[:, c * XCW + j, :]
                        )
                else:
                    nc.sync.dma_start(
                        xch[c][:], xs_r[:, c * XCW : (c + 1) * XCW, :]
                    )
                for wi, ft in pre:
                    wsrc = w1s_r if wi == 0 else w3s_r
                    q = wsp.tile([P, XCW * P], BF16, tag="wq", bufs=12, name="wq")
                    # queue balance: 1.15 MB/chunk on each queue - sync gets
                    # x (0.92) + the last-consumed quarter stream (1,2),
                    # scalar the other 5, both in PE-consumption order.
                    # (sync = x + both ft2 streams was 152 vs 101 GB/s and
                    # the ft2 quarters slipped late every chunk; gpsimd's
                    # queue measured only ~80 GB/s - too slow for any.)
                    qeng = nc.sync if (wi, ft) == (1, NPRE - 1) else nc.scalar
                    qeng.dma_start(
                        q[:], wsrc[:, ft, c * XCW * P : (c + 1) * XCW * P]
                    )
                    wq[(wi, ft, c)] = q
            # biases are tiny (8 KB); ride the otherwise-idle gpsimd queue so
            # they never contend with the x/weight streams. Consumers (the
            # part-A evictions) are emitted below, so Tile sequences them.
            nc.gpsimd.dma_start(b1s[:], b1_r)
            nc.gpsimd.dma_start(b3s[:], b3_r)

            # ---- phase-1 startup matmuls: 4 groups interleaved chunk-wise.
            ps_pre = {}
            for wi, ft in pre:
                ps_pre[(wi, ft)] = psp.tile(
                    [P, T], F32, tag="ps1", bufs=PS1B, name="ps1"
                )
            for c in range(XC):
                for wi, ft in pre:
                    q = wq[(wi, ft, c)]
                    for j in range(XCW):
                        nc.tensor.matmul(
                            ps_pre[(wi, ft)][:],
                            q[:, j * P : (j + 1) * P],
                            xch[c][:, j, :],
                            start=(c == 0 and j == 0),
                            stop=(c == XC - 1 and j == XCW - 1),
                        )

            s1_cur = {}  # ft -> s1 tile

            def evict_p1(wi, ft, ps):
                if wi == 0:
                    s1 = evp.tile([P, T], F32, tag="s1", bufs=2, name="s1")
                    nc.scalar.activation(
                        s1[:], ps[:], silu, bias=b1s[:, ft : ft + 1]
                    )
                    s1_cur[ft] = s1
                else:
                    nc.vector.scalar_tensor_tensor(
                        out=gt[:, ft, :],
                        in0=ps[:],
                        scalar=b3s[:, ft : ft + 1],
                        in1=s1_cur[ft][:],
                        op0=add,
                        op1=mult,
                    )

            for wi, ft in pre:
                evict_p1(wi, ft, ps_pre[(wi, ft)])

            # ---- phase-1 steady state: ft 2..15, full 56-deep groups.
            # ft2/ft3 halves ride directly behind the part-A stream so they
            # land before the PE drains part A; biases after them; b2s later.
            halves = {}
            for ft in range(NPRE, FO):
                for wi in range(2):
                    wsrc = w1s_r if wi == 0 else w3s_r
                    hA = wsp.tile([P, HW_ * P], BF16, tag="wh", bufs=4, name="whA")
                    hB = wsp.tile([P, HW_ * P], BF16, tag="wh", bufs=4, name="whB")
                    nc.sync.dma_start(hA[:], wsrc[:, ft, : HW_ * P])
                    nc.scalar.dma_start(hB[:], wsrc[:, ft, HW_ * P :])
                    halves[(wi, ft)] = (hA, hB)

            for ft in range(NPRE, FO):
                for wi in range(2):
                    hA, hB = halves[(wi, ft)]
                    ps = psp.tile([P, T], F32, tag="ps1", bufs=PS1B, name="ps1")
                    for o in range(DO):
                        if o < HW_:
                            lhsT = hA[:, o * P : (o + 1) * P]
                        else:
                            lhsT = hB[:, (o - HW_) * P : (o - HW_ + 1) * P]
                        nc.tensor.matmul(
                            ps[:],
                            lhsT,
                            xch[o // XCW][:, o % XCW, :],
                            start=(o == 0),
                            stop=(o == DO - 1),
                        )
                    evict_p1(wi, ft, ps)
                # b2 broadcast (3.67 MB) in two 256 KB chunks per late ft
                # group on the scalar queue: the trigger instructions sit
                # behind the silu ACT in the scalar engine's FIFO, so the
                # transfers are time-gated by PE progress and land in the
                # second half of phase 1 where HBM has ~200 GB/s of slack
                # (injecting it into the weight queues at ft==5 caused a
                # 12.6us PE stall + a 20us HAM re-throttle in v2).
                if 8 <= ft <= 14:
                    for cc in (2 * (ft - 8), 2 * (ft - 8) + 1):
                        nc.scalar.dma_start(
                            b2s[:, cc * 512 : (cc + 1) * 512],
                            b2r.ap()[:, cc * 512 : (cc + 1) * 512],
                        )

            # ---------------- phase 2 ----------------
            # BOTH w2 halves ride sync: one queue pumps each ring-gated half
            # at its full ~204 GB/s (per-db delivery 20.6us < 27.6us
            # consumption). Splitting them sync/scalar (tried in v4) made the
            # two ring-gated streams alternate at ~105 GB/s each against the
            # per-core HBM cap, landing each db's hB ~13us late -> one PE
            # stall per db at fo=8 + a HAM re-throttle.
            # The halves split the db block by d (dh), not by fo, and `half`
            # is the OUTER compute loop: hA's last consumer is then half a db
            # (13.8us) before hB's, so the two ring-recycle waits resolve
            # staggered and the sync queue streams continuously (fo-split
            # halves, tried in v5, had both last consumers at the db end ->
            # both next-halves fired together and late -> one stall per db).
            # Output DMAs ride the gpsimd queue (their triggers carry
            # semaphore waits on the DVE adds) so a ring-gated weight DMA
            # can never head-of-line-block them.
            w2h = {}
            for db in range(NDB):
                hA = wsp.tile([P, HFO * DBW], BF16, tag="w2h", bufs=3, name="w2hA")
                hB = wsp.tile([P, HFO * DBW], BF16, tag="w2h", bufs=3, name="w2hB")
                nc.sync.dma_start(hA[:], w2s_r[:, db, : HFO * DBW])
                nc.sync.dma_start(hB[:], w2s_r[:, db, HFO * DBW :])
                w2h[db] = (hA, hB)

            # per-512 PSUM groups (2 banks total) leave room for the 6-bank
            # phase-1 ring; evictions + output DMAs go per 512 columns.
            for db in range(NDB):
                hA, hB = w2h[db]
                for half in range(2):
                    wh = hA if half == 0 else hB
                    dlo = db * DBW + half * 512
                    for ts in range(TT):
                        ps2 = psp.tile([P, 512], F32, tag="ps2", bufs=2, name="ps2")
                        for fo in range(FO):
                            nc.tensor.matmul(
                                ps2[:],
                                gt[:, fo, ts * P : (ts + 1) * P],
                                wh[:, fo * 512 : (fo + 1) * 512],
                                start=(fo == 0),
                                stop=(fo == FO - 1),
                            )
                        ost = evp.tile([P, 512], F32, tag="ost", bufs=3, name="ost")
                        nc.vector.tensor_add(ost[:], ps2[:], b2s[:, dlo : dlo + 512])
                        # outputs alternate gpsimd/scalar (scalar carries
                        # nothing ring-gated in phase 2): gpsimd alone moves
                        # 256 KB in 3.3-5us vs the 3.46us group period, and
                        # the backed-up ost ring was stalling group starts
                        idx = (db * 2 + half) * TT + ts
                        q = nc.gpsimd if idx % 2 == 0 else nc.scalar
                        q.dma_start(out_r[:, ts, dlo : dlo + 512], ost[:])


def build():
    global _NC
    if _NC is not None:
        return _NC
    nc = bacc.Bacc("TRN2", target_bir_lowering=False, debug=False, num_devices=E)
    xs = nc.dram_tensor("xs", [P, DO * T], BF16, kind="ExternalInput")
    w1s = nc.dram_tensor("w1s", [P, FO * DO * P], BF16, kind="ExternalInput")
    w3s = nc.dram_tensor("w3s", [P, FO * DO * P], BF16, kind="ExternalInput")
    w2s = nc.dram_tensor("w2s", [P, NDB * FO * DBW], BF16, kind="ExternalInput")
    b1 = nc.dram_tensor("b1", [F], F32, kind="ExternalInput")
    b3 = nc.dram_tensor("b3", [F], F32, kind="ExternalInput")
    b2r = nc.dram_tensor("b2r", [P, D], F32, kind="ExternalInput")
    out = nc.dram_tensor("out", [T, D], F32, kind="ExternalOutput")
    _emit(nc, xs, w1s, w3s, w2s, b1, b3, b2r, out)
    nc.compile()
    _NC = nc
    return nc


def make_in_maps(x, w1, b1, w3, b3, w2, b2):
    x = np.asarray(x, dtype=np.float32)
    w1 = np.asarray(w1, dtype=np.float32)
    b1 = np.asarray(b1, dtype=np.float32)
    w3 = np.asarray(w3, dtype=np.float32)
    b3 = np.asarray(b3, dtype=np.float32)
    w2 = np.asarray(w2, dtype=np.float32)
    b2 = np.asarray(b2, dtype=np.float32)
    in_maps = []
    for e in range(E):
        # xs[p][o][t] = x[e][t, o*128+p]
        xs = x[e].reshape(T, DO, P).transpose(2, 1, 0).reshape(P, -1).astype(BF)
        # w1s[p][ft][o][fl] = w1[e][ft*128+fl, o*128+p]
        w1s = (
            w1[e].reshape(FO, P, DO, P).transpose(3, 0, 2, 1).reshape(P, -1).astype(BF)
        )
        w3s = (
            w3[e].reshape(FO, P, DO, P).transpose(3, 0, 2, 1).reshape(P, -1).astype(BF)
        )
        # w2s[p][db][dh][fo][dw] = w2[e][db*1024 + dh*512 + dw, fo*128+p]
        # (dh = d-half of the 1024 block: phase-2 `half` loop consumes one
        #  dh-half tile at a time, staggering the two ring-recycle waits)
        w2s = (
            w2[e]
            .reshape(NDB, 2, 512, FO, P)
            .transpose(4, 0, 1, 3, 2)
            .reshape(P, -1)
            .astype(BF)
        )
        in_maps.append(
            {
                "xs": xs,
                "w1s": w1s,
                "w3s": w3s,
                "w2s": w2s,
                "b1": b1[e],
                "b3": b3[e],
                "b2r": np.ascontiguousarray(np.broadcast_to(b2[e], (P, D))),
            }
        )
    return in_maps


def run(x, w1, b1, w3, b3, w2, b2, **spmd_kwargs):
    nc = build()
    in_maps = make_in_maps(x, w1, b1, w3, b3, w2, b2)
    res = run_bass_kernel_spmd(nc, in_maps, core_ids=list(range(E)), **spmd_kwargs)
    out = np.stack([res.results[e]["out"] for e in range(E)], axis=0)
    return out, res


def kernel(x, w1, b1, w3, b3, w2, b2):
    out, _ = run(x, w1, b1, w3, b3, w2, b2)
    return out

